# revision 11
# baseline (speedup 1.0000x reference)
"""Distributed embedding lookup (gather) for 8 Trainium2 NeuronCores.

Strategy (model-parallel, per the row-shard hint):
  - The [1M, 64] f32 table is range-sharded: core c owns rows
    [c*125000, (c+1)*125000)  (32 MB per core, nothing replicated).
  - Host routes each id to its owning core ("all-to-all" of ids done
    host-side), converts to shard-local indices, and buckets them by
    32768-row windows because the on-device gather primitive
    (InstDMAGatherAnt) takes int16 indices.
  - Each bucket is padded to a fixed capacity with index 0 so every
    device-side shape/count is compile-time static (pad slots gather a
    real row that the host ignores).
  - On device each core runs a pipeline of dma_gather (GPSIMD/SWDGE,
    table->SBUF) double-buffered against plain dma_start write-outs
    (sync/HWDGE, SBUF->DRAM).
  - Host scatters the per-core results back into the full
    [16384, 50, 64] output via a slot->original-position map.
"""

import numpy as np

import concourse.bacc as bacc
import concourse.bass as bass
import concourse.mybir as mybir
from concourse.bass_utils import run_bass_kernel_spmd

# ---- problem constants (hardcoded; kernel.py must be self-contained) ----
N_CORES = 8
VOCAB = 1_000_000
EMB = 64                      # 64 f32 = 256 B per row (dma_gather needs %256B)
ROWS_PER_CORE = VOCAB // N_CORES   # 125_000
WIN = 32768                   # int16 index window

# per-core windows: (local_start, height)
WINDOWS = []
_s = 0
while _s < ROWS_PER_CORE:
    WINDOWS.append((_s, min(WIN, ROWS_PER_CORE - _s)))
    _s += WIN
# -> [(0,32768),(32768,32768),(65536,32768),(98304,26696)]

# Fixed per-window slot capacities (multiples of 128).  Expected counts for
# uniform ids: ~26843 per full window, ~21870 for the last -> ~6-7 sigma
# margin; a host-side spill path keeps correctness for any input.
CAPS = [27776, 27776, 27776, 22912]
assert all(c % 128 == 0 for c in CAPS)
CAP_OFFSETS = np.concatenate([[0], np.cumsum(CAPS)]).astype(np.int64)
TOTAL_SLOTS = int(CAP_OFFSETS[-1])          # 106_240
TOTAL_COLS = TOTAL_SLOTS // 16              # idx tensor free dim (int16)

CH_MAX = 8192                 # ids per dma_gather call
NB = 4                        # SBUF destination buffers

# chunks: (window_idx, global_slot_offset, size)
CHUNKS = []
for _w, _cap in enumerate(CAPS):
    _off = int(CAP_OFFSETS[_w])
    _left = _cap
    while _left > 0:
        _sz = min(CH_MAX, _left)
        CHUNKS.append((_w, _off, _sz))
        _off += _sz
        _left -= _sz
assert all(sz % 128 == 0 for _, _, sz in CHUNKS)


def build_nc():
    nc = bacc.Bacc("TRN2")
    shard = nc.dram_tensor(
        "shard", [ROWS_PER_CORE, EMB], mybir.dt.float32, kind="ExternalInput"
    )
    idxs = nc.dram_tensor(
        "idxs", [128, TOTAL_COLS], mybir.dt.int16, kind="ExternalInput"
    )
    out = nc.dram_tensor(
        "out", [TOTAL_SLOTS * EMB], mybir.dt.float32, kind="ExternalOutput"
    )

    from contextlib import ExitStack

    with ExitStack() as stack:
        block = stack.enter_context(nc.Block())
        idx_sb = stack.enter_context(
            nc.sbuf_tensor("idx_sb", [128, TOTAL_COLS], mybir.dt.int16)
        )
        dsts = [
            stack.enter_context(
                nc.sbuf_tensor(f"dst{b}", [128, (CH_MAX // 128) * EMB],
                               mybir.dt.float32)
            )
            for b in range(NB)
        ]
        io_sem = stack.enter_context(nc.semaphore("io"))
        g_sems = [stack.enter_context(nc.semaphore(f"g{b}")) for b in range(NB)]
        o_sems = [stack.enter_context(nc.semaphore(f"o{b}")) for b in range(NB)]

        @block.gpsimd
        def _(gpsimd: bass.BassGpSimd):
            gpsimd.dma_start(idx_sb[:, :], idxs[:, :]).then_inc(io_sem, 16)
            gpsimd.wait_ge(io_sem, 16)
            for i, (w, off, sz) in enumerate(CHUNKS):
                b, r = i % NB, i // NB
                if i >= NB:
                    # wait until the buffer's previous contents were written out
                    gpsimd.wait_ge(o_sems[b], 16 * r)
                wstart, wh = WINDOWS[w]
                dst_ap = dsts[b][:, : (sz // 128) * EMB].rearrange(
                    "p (a e) -> p a e", e=EMB
                )
                gpsimd.dma_gather(
                    dst_ap,
                    shard[wstart : wstart + wh, :],
                    idx_sb[:, off // 16 : (off + sz) // 16],
                    sz,
                    sz,
                    EMB,
                    single_packet=False,  # single-packet caps out ~1-2K idxs
                ).then_inc(g_sems[b], 16)

        @block.sync
        def _(sync: bass.BassEngine):
            uses = [0] * NB
            for i, (w, off, sz) in enumerate(CHUNKS):
                b, r = i % NB, i // NB
                sync.wait_ge(g_sems[b], 16 * (r + 1))
                src = dsts[b][:, : (sz // 128) * EMB]
                dst = out[off * EMB : (off + sz) * EMB].rearrange(
                    "(p f) -> p f", p=128
                )
                sync.dma_start(dst, src).then_inc(o_sems[b], 16)
                uses[b] += 1
            for b in range(NB):
                sync.wait_ge(o_sems[b], 16 * uses[b])

    nc.compile()
    return nc


_NC_CACHE = None
LAST_RESULTS = None  # BassKernelResults of the most recent run (for test.py)
RUN_WALL_S = -1.0    # wall time of the device dispatch+exec (for test.py)


def _get_nc():
    global _NC_CACHE
    if _NC_CACHE is None:
        _NC_CACHE = build_nc()
    return _NC_CACHE


def _route(flat_ids):
    """Route ids to cores/windows/slots.

    Returns (in_maps_idx, slot_pos, spill_pos) where
      in_maps_idx: list of [128, TOTAL_COLS] int16 per core
      slot_pos:    list of [TOTAL_SLOTS] int64 per core (orig flat pos, -1 pad)
      spill_pos:   int64 array of positions handled on host (overflow; ~never)
    """
    owner = flat_ids // ROWS_PER_CORE
    order = np.argsort(owner, kind="stable")
    counts = np.bincount(owner, minlength=N_CORES)
    starts = np.concatenate([[0], np.cumsum(counts)])

    idx_tensors, slot_maps, spill = [], [], []
    for c in range(N_CORES):
        pos_c = order[starts[c] : starts[c + 1]]
        local = flat_ids[pos_c] - c * ROWS_PER_CORE
        w = local // WIN
        worder = np.argsort(w, kind="stable")
        pos_c = pos_c[worder]
        local = local[worder]
        w = w[worder]
        wcounts = np.bincount(w, minlength=len(WINDOWS))
        wstarts = np.concatenate([[0], np.cumsum(wcounts)])

        slot_ids = np.zeros(TOTAL_SLOTS, np.int16)
        slot_pos = np.full(TOTAL_SLOTS, -1, np.int64)
        for wi in range(len(WINDOWS)):
            seg_pos = pos_c[wstarts[wi] : wstarts[wi + 1]]
            seg_li = local[wstarts[wi] : wstarts[wi + 1]] - WINDOWS[wi][0]
            n = len(seg_pos)
            cap = CAPS[wi]
            if n > cap:
                spill.append(seg_pos[cap:])
                seg_pos, seg_li, n = seg_pos[:cap], seg_li[:cap], cap
            base = int(CAP_OFFSETS[wi])
            slot_ids[base : base + n] = seg_li.astype(np.int16)
            slot_pos[base : base + n] = seg_pos

        # per-chunk 16-partition wrap: slot j of a chunk -> [j%16, j//16]
        cols = np.empty((16, TOTAL_COLS), np.int16)
        for _, off, sz in CHUNKS:
            cols[:, off // 16 : (off + sz) // 16] = (
                slot_ids[off : off + sz].reshape(sz // 16, 16).T
            )
        idx_tensors.append(np.tile(cols, (8, 1)))  # replicate to 128 parts
        slot_maps.append(slot_pos)

    spill_pos = (
        np.concatenate(spill) if spill else np.empty(0, np.int64)
    )
    return idx_tensors, slot_maps, spill_pos


def kernel(ids, table):
    ids_np = np.asarray(ids)
    table_np = np.asarray(table, dtype=np.float32)
    flat = ids_np.reshape(-1).astype(np.int64)
    n = flat.shape[0]

    idx_tensors, slot_maps, spill_pos = _route(flat)

    in_maps = [
        {
            "shard": np.ascontiguousarray(
                table_np[c * ROWS_PER_CORE : (c + 1) * ROWS_PER_CORE]
            ),
            "idxs": idx_tensors[c],
        }
        for c in range(N_CORES)
    ]

    nc = _get_nc()
    import time as _time

    _t0 = _time.time()
    res = run_bass_kernel_spmd(nc, in_maps, core_ids=list(range(N_CORES)))
    global LAST_RESULTS, RUN_WALL_S
    RUN_WALL_S = _time.time() - _t0
    LAST_RESULTS = res

    out_flat = np.empty((n, EMB), np.float32)
    for c in range(N_CORES):
        o = np.asarray(res.results[c]["out"]).reshape(-1)
        rows = np.empty((TOTAL_SLOTS, EMB), np.float32)
        for _, off, sz in CHUNKS:
            blk = o[off * EMB : (off + sz) * EMB].reshape(128, sz // 128, EMB)
            rows[off : off + sz] = blk.transpose(1, 0, 2).reshape(sz, EMB)
        valid = slot_maps[c] >= 0
        out_flat[slot_maps[c][valid]] = rows[valid]

    if spill_pos.size:
        out_flat[spill_pos] = table_np[flat[spill_pos]]

    return out_flat.reshape(*ids_np.shape, EMB)
